# revision 2
# baseline (speedup 1.0000x reference)
"""MultiHeadAttention Trainium2 kernel v2 (8-core SPMD, head/tensor parallel).

Problem (hardcoded): stream (2048, 2, 1024) f32, mask (1, 2048, 2048),
w_qkv (1024, 3072), b_qkv (3072,), w_out (1024, 1024), b_out (1024,).
N=2048, B=2, HEADS=16, D_KQ=D_V=64, D_HEAD=192.

Sharding: core d handles batch b=d//4 and heads [4*(d%4), 4*(d%4)+4).
Host sums the 4 partial outputs per batch and adds b_out_eff
(= b_out + b_v @ w_out, because softmax weights sum to 1 the v-bias can be
folded into the output bias on the host).

Per-core dataflow (all bf16 operands, f32 PSUM):
  qkT[f, n]  = w_qkv_loc.T @ x.T          (features on partitions; DVE adds
                                           the per-partition q/k bias)
  v1[m, j, :64] = x @ w_v_loc             (65th column = 1.0, so the PV
                                           matmul's extra output column
                                           accumulates the softmax denom Z)
  per (head j, n-chunk c) over m-tiles:
    lT = k_mt.T @ q_chunk   -> psl (PSUM)         K=64 at tile row 0/64
    wt = exp(lT)            -> SBUF bf16 (Act)
    wt *= exp(maskT) tile   (DVE, all-SBUF bf16)
    psv[n-sub, 0:65] += wt[:, sub].T @ v1[m-tile, j]   (val | Z)
  normalize: val = psv[:, :, :64] * recip(psv[:, :, 64])  (Z is per-partition
             here because n sits on partitions; plain DVE broadcast)
  end: PE-transpose val pairs -> valT[hv, n], outproj nt-tiles -> psum ->
       bf16 SBUF -> DMA out.
"""

import numpy as np
import ml_dtypes

import concourse.tile as tile
from concourse import bacc, mybir
from concourse.bass_utils import run_bass_kernel_spmd

BF16 = ml_dtypes.bfloat16
dt = mybir.dt
AF = mybir.ActivationFunctionType
ALU = mybir.AluOpType

N = 2048
B = 2
DSTR = 1024
HEADS = 16
NH = 4
DKQ = 64
DV = 64
DHEAD = 2 * DKQ + DV
P = 128
KT = DSTR // P        # 8
MT = N // P           # 16
CH = 1024             # attention n-chunk
NCH = N // CH         # 2
N_CORES = 8

f32, bf16 = dt.float32, dt.bfloat16

_BUILT = {}


def _build_nc():
    nc = bacc.Bacc("TRN2", target_bir_lowering=False, debug=False)

    xb = nc.dram_tensor("xb", [P, KT, N], bf16, kind="ExternalInput").ap()
    wqk = nc.dram_tensor("wqk", [P, KT, 4 * P], bf16, kind="ExternalInput").ap()
    wv = nc.dram_tensor("wv", [P, KT, NH * DV], bf16, kind="ExternalInput").ap()
    bqk = nc.dram_tensor("bqk", [P, 4], f32, kind="ExternalInput").ap()
    em = nc.dram_tensor("em", [N, N], bf16, kind="ExternalInput").ap()
    wout = nc.dram_tensor("wout", [P, 2, DSTR], bf16, kind="ExternalInput").ap()
    ident = nc.dram_tensor("ident", [P, P], f32, kind="ExternalInput").ap()
    out = nc.dram_tensor("out", [N, DSTR], bf16, kind="ExternalOutput").ap()

    with tile.TileContext(nc) as tc:
        with (
            tc.tile_pool(name="consts", bufs=1) as consts,
            tc.tile_pool(name="big", bufs=1) as big,
            tc.tile_pool(name="wT", bufs=10) as wT_p,
            tc.tile_pool(name="zr", bufs=2) as zr_p,
            tc.tile_pool(name="ob", bufs=8) as ob_p,
            tc.tile_pool(name="ps", bufs=1, space="PSUM") as ps_p,
        ):
            # ---- persistent SBUF ----
            xb_sb = big.tile([P, KT, N], bf16)
            wqk_sb = big.tile([P, KT, 4 * P], bf16)
            wv_sb = big.tile([P, KT, NH * DV], bf16)
            bqk_sb = consts.tile([P, 4], f32)
            em_sb = big.tile([P, MT, N], bf16)
            qkT = big.tile([P, 4, N], bf16)
            v1 = big.tile([P, MT, NH, 65], bf16)
            val_sb = big.tile([P, MT, NH, DV], f32)
            valT = big.tile([P, 2, N], bf16)
            wout_sb = consts.tile([P, 2, DSTR], bf16)
            id_sb = consts.tile([P, P], f32)

            # ---- input DMAs (ordered for the startup critical path) ----
            # wqk layout groups pair-0 (q01|k01) in cols 0:256 so the first
            # half-DMA unblocks head 0.
            def em_dma(t, c):
                nc.sync.dma_start(out=em_sb[:, t, c * CH:(c + 1) * CH],
                                  in_=em[t * P:(t + 1) * P, c * CH:(c + 1) * CH])

            nc.sync.dma_start(out=bqk_sb, in_=bqk)
            nc.sync.dma_start(out=wqk_sb[:, :, 0:2 * P],
                              in_=wqk[:, :, 0:2 * P])
            NQ = N // 4
            for q in range(2):   # x n-quarters 0,1 (cols 0..1024)
                nc.sync.dma_start(out=xb_sb[:, :, q * NQ:(q + 1) * NQ],
                                  in_=xb[:, :, q * NQ:(q + 1) * NQ])
            nc.sync.dma_start(out=wv_sb, in_=wv.rearrange("p k f -> p (k f)"))
            for t in range(4):
                em_dma(t, 0)
            nc.sync.dma_start(out=xb_sb[:, :, 2 * NQ:3 * NQ],
                              in_=xb[:, :, 2 * NQ:3 * NQ])
            for t in range(4, 10):
                em_dma(t, 0)
            nc.sync.dma_start(out=xb_sb[:, :, 3 * NQ:4 * NQ],
                              in_=xb[:, :, 3 * NQ:4 * NQ])
            for t in range(10, MT):
                em_dma(t, 0)
            for t in range(4):
                em_dma(t, 1)
            nc.sync.dma_start(out=wqk_sb[:, :, 2 * P:4 * P],
                              in_=wqk[:, :, 2 * P:4 * P])
            nc.sync.dma_start(out=id_sb, in_=ident)
            for t in range(4, MT):
                em_dma(t, 1)
            nc.sync.dma_start(out=wout_sb, in_=wout)

            nc.vector.memset(v1[:, :, :, 64:65], 1.0)

            # PSUM budget (8 banks): psl L0/L1 (2 banks each), psv V (2
            # banks, single slot), F x2 slots (1 bank each) shared by the
            # projections, the transposes and the output projection.
            # Projections must never allocate L or V tags while attention
            # holds them, or the tag slot queue deadlocks.
            _rot = [0]

            def ps_tile(shape, dtype, tags=("F",)):
                tag = tags[_rot[0] % len(tags)]
                _rot[0] += 1
                kw = {"bufs": 2} if tag == "F" else {}
                return ps_p.tile(shape, dtype, tag=tag,
                                 name=f"ps{_rot[0]}", **kw)

            # ---- projections (emitted piecewise; see schedule below) ----
            def proj_qk_chunk(ft, nb):
                with nc.named_scope(f"pqk{ft}_{nb}"):
                    ps = ps_tile([P, 512], f32)
                    for kt in range(KT):
                        nc.tensor.matmul(
                            ps,
                            lhsT=wqk_sb[:, kt, ft * P:(ft + 1) * P],
                            rhs=xb_sb[:, kt, nb * 512:(nb + 1) * 512],
                            start=(kt == 0), stop=(kt == KT - 1),
                        )
                    nc.vector.tensor_scalar(
                        qkT[:, ft, nb * 512:(nb + 1) * 512], ps,
                        bqk_sb[:, ft:ft + 1], None, ALU.add,
                    )

            def proj_v_pair(mt):
                # two m-tiles share one F-slot psum to halve the F queue
                with nc.named_scope(f"pv{mt}"):
                    ps = ps_tile([P, 2 * NH * DV], f32)
                    for half in range(2):
                        m = mt + half
                        for kt in range(KT):
                            nc.tensor.matmul(
                                ps[:, half * 256:(half + 1) * 256],
                                lhsT=xb_sb[:, kt, m * P:(m + 1) * P],
                                rhs=wv_sb[:, kt, :],
                                start=(kt == 0), stop=(kt == KT - 1),
                            )
                    nc.vector.tensor_copy(
                        out=v1[:, mt:mt + 2, :, 0:DV],
                        in_=ps.rearrange("p (h j d) -> p h j d", j=NH, d=DV),
                    )

            # ---- attention for one head over one n-chunk ----
            def attn(j, c, inject=None):
                p, oe = j // 2, j % 2
                base = 64 * oe
                with nc.named_scope(f"attn_j{j}_c{c}"):
                    psv = ps_p.tile([P, 8, P], f32, tag="V",
                                    name=f"psv_j{j}_c{c}")
                    for mt in range(MT):
                        psl = ps_p.tile([P, CH], f32, tag="L" + str(mt % 2))
                        for h in range(2):
                            nc.tensor.matmul(
                                psl[:, h * 512:(h + 1) * 512],
                                lhsT=qkT[base:base + 64, 2 * p + 1,
                                         mt * P:(mt + 1) * P],
                                rhs=qkT[base:base + 64, 2 * p,
                                        c * CH + h * 512:c * CH + (h + 1) * 512],
                                start=True, stop=True,
                            )
                        wt = wT_p.tile([P, CH], bf16)
                        nc.scalar.activation(out=wt, in_=psl, func=AF.Exp)
                        nc.vector.tensor_mul(
                            out=wt, in0=wt, in1=em_sb[:, mt, c * CH:(c + 1) * CH])
                        for sub in range(8):
                            nc.tensor.matmul(
                                psv[:, sub, 0:65],
                                lhsT=wt[:, sub * P:(sub + 1) * P],
                                rhs=v1[:, mt, j, :],
                                # start marks the whole PSUM bank pending-zero,
                                # so only the first sub of each bank may set it
                                start=(mt == 0 and sub % 4 == 0),
                                stop=(mt == MT - 1),
                                skip_group_check=True,
                            )
                        if inject is not None:
                            inject(mt)
                    with nc.named_scope(f"norm_j{j}_c{c}"):
                        zr = zr_p.tile([P, 8], f32)
                        nc.vector.reciprocal(out=zr, in_=psv[:, :, 64])
                        nc.vector.tensor_mul(
                            out=val_sb[:, c * 8:(c + 1) * 8, j, :],
                            in0=psv[:, :, 0:DV],
                            in1=zr.unsqueeze(-1).broadcast_to([P, 8, DV]),
                        )

            # ---- tail: transpose val -> valT, output projection ----
            def trans(p, nt, tags=("F",)):
                with nc.named_scope(f"tr{p}_{nt}"):
                    pst = ps_tile([P, P], f32, tags)
                    nc.tensor.transpose(
                        pst,
                        val_sb[:, nt, 2 * p:2 * p + 2, :],
                        id_sb,
                    )
                    nc.vector.tensor_copy(
                        out=valT[:, p, nt * P:(nt + 1) * P], in_=pst)

            def outproj(nt, copy_eng, tags=("F",)):
                with nc.named_scope(f"fin{nt}"):
                    for dc in range(2):
                        po = ps_tile([P, 512], f32, tags)
                        for i in range(2):
                            nc.tensor.matmul(
                                po,
                                lhsT=valT[:, i, nt * P:(nt + 1) * P],
                                rhs=wout_sb[:, i, dc * 512:(dc + 1) * 512],
                                start=(i == 0), stop=(i == 1),
                            )
                        ob = ob_p.tile([P, 512], bf16)
                        if copy_eng == "act":
                            nc.scalar.copy(out=ob, in_=po)
                        else:
                            nc.vector.tensor_copy(out=ob, in_=po)
                        nc.sync.dma_start(
                            out=out[nt * P:(nt + 1) * P,
                                    dc * 512:(dc + 1) * 512],
                            in_=ob)

            # ---- schedule ----
            # ft order: 0 = q pair0, 1 = k pair0, 2 = q pair1, 3 = k pair1.
            # Dependencies are tracked by emission order only (no forward
            # deps), so every producer is EMITTED before its first consumer.
            # Work that should merely fill PE gaps (remaining projections,
            # transposes) is emitted early but wrapped in
            # tc.high_priority(offset=-LATE), which makes the scheduler
            # treat it as if it were issued much later: the attention
            # critical path (QK -> exp -> mult -> PV) always wins the heap.
            LATE = -(10 ** 6)

            # prologue: exactly what head 0 / chunk 0 needs first
            proj_qk_chunk(0, 0)
            proj_qk_chunk(1, 0)
            proj_qk_chunk(0, 1)

            # F-slot queue drains in emission order -> order fillers by the
            # deadline of their first consumer.
            with tc.high_priority(offset=LATE):
                proj_v_pair(0)
                proj_qk_chunk(1, 1)
                proj_v_pair(2)
                proj_v_pair(4)
                proj_qk_chunk(1, 2)
                proj_v_pair(6)
                proj_qk_chunk(1, 3)
                proj_v_pair(8)
                proj_qk_chunk(0, 2)
                proj_qk_chunk(0, 3)
                proj_v_pair(10)
                proj_v_pair(12)
                proj_v_pair(14)

            attn(0, 0)
            with tc.high_priority(offset=LATE):
                for ft in (2, 3):
                    for nb in range(4):
                        proj_qk_chunk(ft, nb)
            attn(1, 0)
            with tc.high_priority(offset=LATE):
                for nt in range(8):
                    trans(0, nt)
            attn(0, 1)
            attn(1, 1)
            with tc.high_priority(offset=LATE):
                for nt in range(8, MT):
                    trans(0, nt)
            attn(2, 0)
            attn(3, 0)
            with tc.high_priority(offset=LATE):
                for nt in range(8):
                    trans(1, nt)
                for nt in range(8):
                    outproj(nt, "vec")
            attn(2, 1)
            attn(3, 1)
            TAILTAGS = ("L0", "L1", "V", "F")
            for nt in range(8, MT):
                trans(1, nt, TAILTAGS)
            for nt in range(8, MT):
                outproj(nt, "act" if nt % 2 else "vec", TAILTAGS)

    nc.compile()
    return nc


def get_nc():
    if "nc" not in _BUILT:
        _BUILT["nc"] = _build_nc()
    return _BUILT["nc"]


def _shard_inputs(stream, mask, w_qkv, b_qkv, w_out):
    stream = np.asarray(stream, np.float32)
    mask = np.asarray(mask, np.float32)
    w_qkv = np.asarray(w_qkv, np.float32)
    b_qkv = np.asarray(b_qkv, np.float32)
    w_out = np.asarray(w_out, np.float32)

    em = np.exp(mask[0].T).astype(BF16)
    ident = np.eye(P, dtype=np.float32)
    xbs = []
    for b in range(B):
        xT = stream[:, b, :].T  # (1024, 2048)
        xbs.append(np.ascontiguousarray(
            xT.reshape(KT, P, N).transpose(1, 0, 2)).astype(BF16))

    in_maps = []
    for d in range(N_CORES):
        b = d // 4
        heads = [(d % 4) * 4 + j for j in range(NH)]
        qc = [w_qkv[:, h * DHEAD:h * DHEAD + DKQ] for h in heads]
        kc = [w_qkv[:, h * DHEAD + DKQ:h * DHEAD + 2 * DKQ] for h in heads]
        vc = [w_qkv[:, h * DHEAD + 2 * DKQ:(h + 1) * DHEAD] for h in heads]
        wqk = np.concatenate(
            [qc[0], qc[1], kc[0], kc[1], qc[2], qc[3], kc[2], kc[3]], axis=1)
        wqk = np.ascontiguousarray(
            wqk.reshape(KT, P, 4 * P).transpose(1, 0, 2)).astype(BF16)
        wvd = np.concatenate(vc, axis=1)
        wvd = np.ascontiguousarray(
            wvd.reshape(KT, P, NH * DV).transpose(1, 0, 2)).astype(BF16)
        bq = [b_qkv[h * DHEAD:h * DHEAD + DKQ] for h in heads]
        bk = [b_qkv[h * DHEAD + DKQ:h * DHEAD + 2 * DKQ] for h in heads]
        bqk_arr = np.stack(
            [np.concatenate([bq[0], bq[1]]), np.concatenate([bk[0], bk[1]]),
             np.concatenate([bq[2], bq[3]]), np.concatenate([bk[2], bk[3]])],
            axis=1).astype(np.float32)
        woutd = np.concatenate(
            [w_out[h * DV:(h + 1) * DV, :] for h in heads], axis=0)
        woutd = np.ascontiguousarray(
            woutd.reshape(2, P, DSTR).transpose(1, 0, 2)).astype(BF16)
        in_maps.append({
            "xb": xbs[b], "wqk": wqk, "wv": wvd, "bqk": bqk_arr,
            "em": em, "wout": woutd, "ident": ident,
        })
    return in_maps


def kernel(stream, mask, w_qkv, b_qkv, w_out, b_out):
    nc = get_nc()
    in_maps = _shard_inputs(stream, mask, w_qkv, b_qkv, w_out)
    res = run_bass_kernel_spmd(nc, in_maps, core_ids=list(range(N_CORES)))
    b_qkv = np.asarray(b_qkv, np.float32)
    w_out = np.asarray(w_out, np.float32)
    bv = np.concatenate(
        [b_qkv[h * DHEAD + 2 * DKQ:(h + 1) * DHEAD] for h in range(HEADS)])
    b_out_eff = np.asarray(b_out, np.float32) + bv @ w_out
    out = np.empty((N, B, DSTR), np.float32)
    for b in range(B):
        acc = res.results[4 * b]["out"].astype(np.float32)
        for i in range(1, 4):
            acc += res.results[4 * b + i]["out"].astype(np.float32)
        out[:, b, :] = acc + b_out_eff
    return out
